# revision 42
# baseline (speedup 1.0000x reference)
"""Expert-parallel MoE (top-1 routing, SwiGLU experts) for 8 Trainium2 cores.

Strategy (matches the expert-parallel sharding hint):
  - Router (4096x1024 @ 1024x8 matmul + softmax + argmax) runs on host as
    part of the sharding step: tokens are dispatched to the core owning
    their top-1 expert.
  - Core e holds Wu[e], Wv[e], Wd[e] and computes SwiGLU for only the
    tokens routed to expert e (dropless MoE: sum over experts of
    gate * ye is exactly top1_p * y_top1 since the one-hot gate zeroes
    every other expert).
  - Tokens ship transposed (d on partitions) so both projections use the
    native weight layouts as the stationary matmul operand and no
    on-device transposes are needed.  Output returns as y^T [d, C]; host
    scales by the top-1 probability and scatters back.
  - Matmuls run in float32r (full 78.6 TF/s PE rate; fp32 storage,
    reduced-precision multiply) with fp32 PSUM accumulation.

Per-core kernel (capacity C tokens, padded with zeros):
  phase A: u^T/v^T tiles [128h, Ct] = sum_d Wu/Wv tile.T @ x^T tile,
           h^T = silu(u^T) * v^T kept resident in SBUF [2752, C]
  phase B: y^T tiles [128d, Ct] = sum_h Wd tile.T @ h^T tile -> DRAM
"""

import sys

import numpy as np

try:  # concourse ships on PYTHONPATH in the runtime image; add fallbacks
    import concourse  # noqa: F401
except ImportError:  # pragma: no cover
    for p in ("/opt/trn_rl_repo", "/opt/pypackages",
              "/root/.axon_site/_ro/trn_rl_repo", "/root/.axon_site/_ro/pypackages"):
        if p not in sys.path:
            sys.path.append(p)

D_MODEL = 1024
N_EXPERTS = 8
HIDDEN = 2752
ALPHA = 0.05
N_CORES = 8
DT = D_MODEL // 128           # 8 d-tiles
H_TILES = [(i * 128, min(128, HIDDEN - i * 128)) for i in range((HIDDEN + 127) // 128)]
CHUNK_MAX = 640               # max tokens per expert per launch (SBUF budget)

_CACHE = {}

# tuning knobs (baked into the compiled kernel; tuned via TimelineSim)
CFG = {
    "wuv_bufs": 4,                # phase-A weight-stream double buffering
    "wd_bufs": 3,                 # phase-B weight-stream prefetch depth
    "store_alt": True,            # alternate yt stores across both HWDGE rings
    "block_ht": 2,                # h-tiles per steady-state weight block
    "packed_tail": True,          # pack Wu|Wv tail into one stationary operand
    "b0_ci_outer": True,          # block-0 emitted ci-outer w/ xt col-split
}


def _col_tiles(C):
    """Split C columns into tiles <=512, each >=256 when C allows (float32r
    runs at full PE rate only when the moving free dim is >=256)."""
    n = max(1, -(-C // 512))
    base = -(-C // n)
    base = -(-base // 32) * 32
    tiles = []
    off = 0
    while off < C:
        w = min(base, C - off)
        tiles.append((off, w))
        off += w
    return tiles


def _build(C):
    import concourse.bacc as bacc
    import concourse.mybir as mybir
    import concourse.tile as tile

    f32 = mybir.dt.float32
    f32r = mybir.dt.float32r
    CT = _col_tiles(C)

    nc = bacc.Bacc("TRN2", target_bir_lowering=False, debug=False,
                   num_devices=N_CORES)
    xt_d = nc.dram_tensor("xt", (D_MODEL, C), f32r, kind="ExternalInput").ap()
    wu_d = nc.dram_tensor("wu", (D_MODEL, HIDDEN), f32r, kind="ExternalInput").ap()
    wv_d = nc.dram_tensor("wv", (D_MODEL, HIDDEN), f32r, kind="ExternalInput").ap()
    wd_d = nc.dram_tensor("wd", (HIDDEN, D_MODEL), f32r, kind="ExternalInput").ap()
    yt_d = nc.dram_tensor("yt", (D_MODEL, C), f32, kind="ExternalOutput").ap()

    # group the 21 full h-tiles into >=1MiB weight-stream blocks (block_ht
    # tiles each); the 64-row tail tile is handled separately with Wu|Wv
    # packed into one stationary operand (see below)
    full_tiles = H_TILES[:21] if CFG["packed_tail"] else H_TILES
    tail_off, tail_w = H_TILES[21]
    blocks = []
    i = 0
    while i < len(full_tiles):
        n = 2 if i == 0 else CFG["block_ht"]
        pair = full_tiles[i:i + n]
        off = pair[0][0]
        width = sum(w for _, w in pair)
        blocks.append((off, width, pair))
        i += len(pair)

    with tile.TileContext(nc) as tc:
        with (
            tc.tile_pool(name="xt", bufs=1) as xt_pool,
            tc.tile_pool(name="h", bufs=1) as h_pool,
            tc.tile_pool(name="wuv", bufs=CFG["wuv_bufs"]) as wuv_pool,
            tc.tile_pool(name="wd", bufs=CFG["wd_bufs"]) as wd_pool,
            tc.tile_pool(name="tmp", bufs=4) as tmp_pool,
            tc.tile_pool(name="out", bufs=4) as out_pool,
            tc.tile_pool(name="ps", bufs=2, space="PSUM") as ps_pool,
        ):
            xt = xt_pool.tile([128, DT, C], f32r)
            xt_src = xt_d.rearrange("(t p) c -> p t c", p=128)
            h = h_pool.tile([128, len(H_TILES), C], f32r)

            # ---- phase A: h^T = silu(x^T Wu) * (x^T Wv), H on partitions
            for bi, (boff, bw, pair) in enumerate(blocks):
                wu_t = wuv_pool.tile([128, DT, bw], f32r, tag="wu")
                wu_src = wu_d[:, boff:boff + bw].rearrange("(t p) h -> p t h", p=128)
                wv_t = wuv_pool.tile([128, DT, bw], f32r, tag="wv")
                wv_src = wv_d[:, boff:boff + bw].rearrange("(t p) h -> p t h", p=128)
                def emit_group(hoff, hw, cs, cw, loc):
                    hi = hoff // 128
                    pu = ps_pool.tile([128, cw], f32, tag="psu")
                    for di in range(DT):
                        nc.tensor.matmul(
                            pu[:hw, :cw], wu_t[:, di, loc:loc + hw],
                            xt[:, di, cs:cs + cw],
                            start=(di == 0), stop=(di == DT - 1))
                    pv = ps_pool.tile([128, cw], f32, tag="psv")
                    for di in range(DT):
                        nc.tensor.matmul(
                            pv[:hw, :cw], wv_t[:, di, loc:loc + hw],
                            xt[:, di, cs:cs + cw],
                            start=(di == 0), stop=(di == DT - 1))
                    sl = tmp_pool.tile([128, cw], f32, tag="silu")
                    nc.scalar.activation(sl[:hw, :cw], pu[:hw, :cw],
                                         mybir.ActivationFunctionType.Silu)
                    nc.vector.tensor_mul(h[:hw, hi, cs:cs + cw],
                                         sl[:hw, :cw], pv[:hw, :cw])

                if bi == 0 and CFG["b0_ci_outer"]:
                    # Initial fill, interleaved in first-consumption order so
                    # the first u-matmuls start after ~1 MB instead of the
                    # whole 4.3 MB working set.  ~0.5-1 MB chunks: finer ones
                    # are HWDGE-dispatch-bound (~650 ns sequencer per DMA).
                    # xt is split by column tile too, and the compute below is
                    # emitted ci-outer, so the ci1 half of xt is off the
                    # critical path.
                    cs0, cw0 = CT[0]
                    nc.sync.dma_start(wu_t[:, 0:4, :], wu_src[:, 0:4, :])
                    nc.sync.dma_start(xt[:, 0:4, cs0:cs0 + cw0],
                                      xt_src[:, 0:4, cs0:cs0 + cw0])
                    nc.sync.dma_start(wu_t[:, 4:8, :], wu_src[:, 4:8, :])
                    nc.sync.dma_start(xt[:, 4:8, cs0:cs0 + cw0],
                                      xt_src[:, 4:8, cs0:cs0 + cw0])
                    nc.sync.dma_start(wv_t[:, 0:4, :], wv_src[:, 0:4, :])
                    nc.sync.dma_start(wv_t[:, 4:8, :], wv_src[:, 4:8, :])
                    for cs, cw in CT[1:]:
                        nc.sync.dma_start(xt[:, 0:4, cs:cs + cw],
                                          xt_src[:, 0:4, cs:cs + cw])
                        nc.sync.dma_start(xt[:, 4:8, cs:cs + cw],
                                          xt_src[:, 4:8, cs:cs + cw])
                    for cs, cw in CT:
                        for hoff, hw in pair:
                            emit_group(hoff, hw, cs, cw, hoff - boff)
                else:
                    if bi == 0:
                        nc.sync.dma_start(wu_t[:, 0:4, :], wu_src[:, 0:4, :])
                        nc.sync.dma_start(xt[:, 0:4, :], xt_src[:, 0:4, :])
                        nc.sync.dma_start(wu_t[:, 4:8, :], wu_src[:, 4:8, :])
                        nc.sync.dma_start(xt[:, 4:8, :], xt_src[:, 4:8, :])
                        nc.sync.dma_start(wv_t[:, 0:4, :], wv_src[:, 0:4, :])
                        nc.sync.dma_start(wv_t[:, 4:8, :], wv_src[:, 4:8, :])
                    else:
                        nc.sync.dma_start(wu_t[:], wu_src)
                        nc.sync.dma_start(wv_t[:], wv_src)
                    for hoff, hw in pair:
                        for cs, cw in CT:
                            emit_group(hoff, hw, cs, cw, hoff - boff)

            # ---- phase A tail (64 h-rows): Wu|Wv packed side by side into a
            # single stationary operand -> one 8-matmul chain produces u in
            # psum partitions 0:64 and v in 64:128 (halves the tail matmuls)
            if CFG["packed_tail"]:
                wt_tail = wuv_pool.tile([128, DT, 128], f32r, tag="wtail")
                tl_u = wu_d[:, tail_off:tail_off + tail_w].rearrange(
                    "(t p) h -> p t h", p=128)
                tl_v = wv_d[:, tail_off:tail_off + tail_w].rearrange(
                    "(t p) h -> p t h", p=128)
                nc.sync.dma_start(wt_tail[:, :, 0:tail_w], tl_u)
                nc.sync.dma_start(wt_tail[:, :, tail_w:2 * tail_w], tl_v)
                for cs, cw in CT:
                    puv = ps_pool.tile([128, cw], f32, tag="psu")
                    for di in range(DT):
                        nc.tensor.matmul(
                            puv[:, :cw], wt_tail[:, di, :],
                            xt[:, di, cs:cs + cw],
                            start=(di == 0), stop=(di == DT - 1))
                    sl = tmp_pool.tile([128, cw], f32, tag="silu")
                    nc.scalar.activation(sl[:tail_w, :cw], puv[:tail_w, :cw],
                                         mybir.ActivationFunctionType.Silu)
                    nc.vector.tensor_mul(h[:tail_w, 21, cs:cs + cw],
                                         sl[:tail_w, :cw],
                                         puv[tail_w:2 * tail_w, :cw])

            # ---- phase B: y^T = h^T.T-contracted with Wd, D on partitions
            for dj in range(DT):
                wd_t = wd_pool.tile([128, len(H_TILES), 128], f32r, tag="wd")
                nc.sync.dma_start(
                    wd_t[:, :21, :],
                    wd_d[:2688, dj * 128:(dj + 1) * 128].rearrange(
                        "(t p) d -> p t d", p=128))
                nc.sync.dma_start(
                    wd_t[:64, 21, :], wd_d[2688:HIDDEN, dj * 128:(dj + 1) * 128])
                for cs, cw in CT:
                    py = ps_pool.tile([128, cw], f32, tag="psy")
                    for hi, (hoff, hw) in enumerate(H_TILES):
                        nc.tensor.matmul(
                            py[:, :cw], wd_t[:hw, hi, :],
                            h[:hw, hi, cs:cs + cw],
                            start=(hi == 0), stop=(hi == len(H_TILES) - 1))
                    ot = out_pool.tile([128, cw], f32, tag="yt")
                    nc.vector.tensor_copy(ot[:, :cw], py[:, :cw])
                    eng = nc.sync if (CFG["store_alt"] and dj % 2) else nc.scalar
                    eng.dma_start(yt_d[dj * 128:(dj + 1) * 128, cs:cs + cw],
                                  ot[:, :cw])
    nc.compile()
    return nc


def _get_kernel(C):
    if C not in _CACHE:
        _CACHE[C] = _build(C)
    return _CACHE[C]


def kernel(x, Wg, bg, Wu, Wv, Wd):
    from concourse.bass_utils import run_bass_kernel_spmd

    x = np.ascontiguousarray(np.asarray(x, dtype=np.float32))
    Wg = np.asarray(Wg, dtype=np.float32)
    bg = np.asarray(bg, dtype=np.float32)
    Wu = np.asarray(Wu, dtype=np.float32)
    Wv = np.asarray(Wv, dtype=np.float32)
    Wd = np.asarray(Wd, dtype=np.float32)

    b, t, d = x.shape
    N = b * t
    xf = x.reshape(N, d)

    # ---- host router (the sharding step): top-1 expert per token
    logits = xf.astype(np.float64) @ Wg.astype(np.float64) + bg.astype(np.float64)
    lmax = logits.max(axis=-1, keepdims=True)
    e = np.exp(logits - lmax)
    probs = e / e.sum(axis=-1, keepdims=True)          # float64
    top1 = np.argmax(logits, axis=-1)
    top1_p = probs[np.arange(N), top1].astype(np.float32)

    # aux load-balancing loss (scalar)
    me = np.bincount(top1, minlength=N_EXPERTS).astype(np.float64) / N
    ce = probs.mean(axis=0)
    aux = np.float32(ALPHA * N_EXPERTS * float(np.sum(me * ce)))

    idx = [np.nonzero(top1 == ei)[0] for ei in range(N_EXPERTS)]
    counts = np.array([len(i) for i in idx])

    y = np.zeros((N, d), dtype=np.float32)
    n_chunks = max(1, -(-int(counts.max()) // CHUNK_MAX))
    for ch in range(n_chunks):
        sub = [i[ch * CHUNK_MAX:(ch + 1) * CHUNK_MAX] for i in idx]
        cmax = max(len(s) for s in sub)
        if cmax == 0:
            continue
        C = max(320, -(-cmax // 32) * 32)
        nc = _get_kernel(C)
        in_maps = []
        for ei in range(N_EXPERTS):
            xt = np.zeros((d, C), dtype=np.float32)
            ns = len(sub[ei])
            if ns:
                xt[:, :ns] = xf[sub[ei]].T
            in_maps.append({"xt": xt, "wu": Wu[ei], "wv": Wv[ei], "wd": Wd[ei]})
        res = None
        for attempt in range(3):
            try:
                res = run_bass_kernel_spmd(nc, in_maps, list(range(N_CORES)))
                break
            except Exception:
                if attempt == 2:
                    raise
                import time
                time.sleep(5.0)
        for ei in range(N_EXPERTS):
            ns = len(sub[ei])
            if ns:
                yt = res.results[ei]["yt"]
                y[sub[ei]] = top1_p[sub[ei], None] * yt[:, :ns].T

    return y.reshape(b, t, d), aux
